# revision 5
# baseline (speedup 1.0000x reference)
"""Trainium2 Bass kernel for nn_CVAE decoder LSTM (v2).

Data-parallel over 8 NeuronCores: batch 8192 -> 1024 per core.

Math (per core, B=1024, T=2048, H=128):
  W_comb = W_hh.T + W_out.T @ Wx.T            [128, 512]
  P0     = static_proj + b_out @ Wx.T         [B, 512]  (constant per step)
  gates_t = h_t @ W_comb + P0                 (t >= 1)
  gates_0 = static_proj + start @ Wx.T = Pf   (h_0 = 0)

v2 layout: hid-on-partition, batch on free. NG=2 groups of BG=512.
Gate order permuted to [i, f, o, g]. Per group per step two PSUM tiles
t_if [128,2,512] and t_og [128,2,512]; each 1-bank gate slice gets
ident@P0 (start) then W@h (accumulate, stop) - idents issued first so
they run during the h dependency window. x' = h @ WoutT accumulated
directly into a 1-bank PSUM tile x_ps[128, U, 8, 2] across a U-step
chunk; one bias-add + one DMA per chunk.

Engines: ACT sig(if) sig(o) tanh(g) tanh(c); DVE t2=f*c, c'=t2+u,
h=o*tau; Pool u=i*g.
"""

import os
import numpy as np

import concourse.bass as bass
import concourse.bacc as bacc
import concourse.tile as tile
from concourse import mybir
from concourse.bass_utils import run_bass_kernel_spmd

f32 = mybir.dt.float32
f32r = mybir.dt.float32r
AF = mybir.ActivationFunctionType

HIDDEN = 128
INPUT_SIZE = 2
T = 2048
B_FULL = 8192
NCORES = 8
BC = B_FULL // NCORES      # 1024 batch rows per core
NG = 2                     # batch groups per core
BG = BC // NG              # 512 rows per group
NBT = BG // 128            # 4 batch-tiles of 128 per group
KT = NG * NBT              # 8 batch-tiles of 128 per core
U = int(os.environ.get("K_U", "32"))   # steps per loop chunk (x_ps: U*64B <= 2KB)
SKIPGC = os.environ.get("K_SKIPGC", "1") == "1"


def _build_nc(t_total=T):
    nc = bacc.Bacc("TRN2", target_bir_lowering=False)

    p0_d = nc.dram_tensor("p0", [4, HIDDEN, BC], f32r, kind="ExternalInput")
    pf_d = nc.dram_tensor("pf", [4, HIDDEN, BC], f32, kind="ExternalInput")
    wcomb_d = nc.dram_tensor("wcomb", [HIDDEN, 4 * HIDDEN], f32r, kind="ExternalInput")
    woutT_d = nc.dram_tensor("woutT", [HIDDEN, INPUT_SIZE], f32r, kind="ExternalInput")
    ident_d = nc.dram_tensor("ident", [HIDDEN, HIDDEN], f32r, kind="ExternalInput")
    bout_d = nc.dram_tensor("bout", [1, U * KT * INPUT_SIZE], f32,
                            kind="ExternalInput")
    y_d = nc.dram_tensor("y", [BC, t_total, INPUT_SIZE], f32, kind="ExternalOutput")
    # [p, k, t, j] view of y: batch row b = k*128 + p
    y_v = y_d.ap().rearrange("(k p) t j -> p k t j", p=128)

    u0 = min(U, t_total)
    n_chunks = t_total // U if t_total >= U else 1

    with tile.TileContext(nc) as tc:
        with (
            tc.tile_pool(name="consts", bufs=1) as consts,
            tc.tile_pool(name="hpool", bufs=2 * NG) as hpool,
            tc.tile_pool(name="cpool", bufs=2 * NG) as cpool,
            tc.tile_pool(name="cell", bufs=4 * NG) as cell,
            tc.tile_pool(name="ybuf", bufs=2) as ybuf,
            tc.tile_pool(name="ps_g", bufs=1, space="PSUM") as ps_g,
            tc.tile_pool(name="ps_x", bufs=1, space="PSUM") as ps_x,
        ):
            # ---- constants ----
            wcomb = consts.tile([HIDDEN, 4 * HIDDEN], f32r)
            woutT = consts.tile([HIDDEN, INPUT_SIZE], f32r)
            ident = consts.tile([HIDDEN, HIDDEN], f32r)
            bout = consts.tile([128, U, KT * INPUT_SIZE], f32)
            p0 = consts.tile([128, 4, BC], f32r)   # [hid, gate, batch]
            nc.gpsimd.dma_start(out=wcomb, in_=wcomb_d[:, :])
            nc.gpsimd.dma_start(out=woutT, in_=woutT_d[:, :])
            nc.gpsimd.dma_start(out=ident, in_=ident_d[:, :])
            nc.gpsimd.dma_start(
                out=bout,
                in_=bout_d.ap().rearrange("o (u x) -> o u x", u=U)
                .to_broadcast((128, U, KT * INPUT_SIZE)))
            nc.gpsimd.dma_start(out=p0, in_=p0_d.ap().rearrange("g p b -> p g b"))

            def step(gi, h_prev, c_prev, x_ps, s, pf_sb=None):
                """One LSTM cell step for group gi. Returns (h_new, c_new)."""
                bs = slice(gi * BG, (gi + 1) * BG)
                if pf_sb is None:
                    t_if = ps_g.tile([128, 2, BG], f32, tag="t_if", bufs=2)
                    t_o = ps_g.tile([128, BG], f32, tag="t_o", bufs=2)
                    t_g = ps_g.tile([128, BG], f32, tag="t_g", bufs=1)
                    # gate slice -> (out AP, P0 row). t_g first: it is
                    # single-buffered, free it earliest. Idents (start) are
                    # issued before W@h so they run in the h-dep window.
                    slots = [(t_g, 3), (t_if[:, 0, :], 0), (t_if[:, 1, :], 1),
                             (t_o, 2)]
                    for out_ap, gg in slots:
                        nc.tensor.matmul(
                            out_ap, ident, p0[:, gg, bs],
                            start=True, stop=False, skip_group_check=SKIPGC)
                    for out_ap, gg in slots:
                        nc.tensor.matmul(
                            out_ap,
                            wcomb[:, gg * 128:(gg + 1) * 128],
                            h_prev[:, :],
                            start=False, stop=True, skip_group_check=SKIPGC)
                else:
                    t_if = pf_sb[:, 0:2, bs]
                    t_o = pf_sb[:, 2, bs]
                    t_g = pf_sb[:, 3, bs]

                # activations: tanh(g) | sig(i,f) | sig(o)
                g_t = cell.tile([128, BG], f32, tag="g_t")
                nc.scalar.activation(g_t, t_g, AF.Tanh)
                s_if = cell.tile([128, 2, BG], f32, tag="s_if")
                nc.scalar.activation(s_if, t_if, AF.Sigmoid)
                s_o = cell.tile([128, BG], f32, tag="s_o")
                nc.scalar.activation(s_o, t_o, AF.Sigmoid)

                # u = sig(i)*tanh(g) on Pool; t2 = sig(f)*c on DVE
                u = cell.tile([128, BG], f32, tag="u")
                nc.gpsimd.tensor_mul(u, s_if[:, 0, :], g_t)
                t2 = cell.tile([128, BG], f32, tag="t2")
                nc.vector.tensor_mul(t2, s_if[:, 1, :], c_prev)
                c_new = cpool.tile([128, BG], f32)
                nc.vector.tensor_add(c_new, t2, u)

                tau = cell.tile([128, BG], f32, tag="tau")
                nc.scalar.activation(tau, c_new, AF.Tanh)
                h_new = hpool.tile([128, BG], f32r)
                nc.vector.tensor_mul(h_new, s_o, tau)

                # x' = h' @ WoutT into the chunk PSUM accumulator
                for k in range(NBT):
                    nc.tensor.matmul(
                        x_ps[:, s, NBT * gi + k, :],
                        h_new[:, k * 128:(k + 1) * 128],
                        woutT,
                        start=True, stop=True)
                return h_new, c_new

            def run_chunk(hs, cs, t_slice, first=False):
                x_ps_t = ps_x.tile([128, U, KT * INPUT_SIZE], f32)
                x_ps = x_ps_t.rearrange("p u (k j) -> p u k j", j=INPUT_SIZE)
                for s in range(U):
                    new = [step(gi, hs[gi], cs[gi], x_ps, s,
                                pf_sb=(pf_sb if (first and s == 0) else None))
                           for gi in range(NG)]
                    hs = [n[0] for n in new]
                    cs = [n[1] for n in new]
                y_sb = ybuf.tile([128, KT, U, INPUT_SIZE], f32)
                nc.vector.tensor_add(
                    y_sb.rearrange("p k u j -> p u k j"),
                    x_ps,
                    bout.rearrange("p u (k j) -> p u k j", j=INPUT_SIZE))
                nc.sync.dma_start(out=y_v[:, :, t_slice, :], in_=y_sb)
                return hs, cs

            # ---- initial state ----
            cs = []
            for gi in range(NG):
                c0 = cpool.tile([128, BG], f32, name="c0")
                nc.vector.memset(c0, 0.0)
                cs.append(c0)
            hs = [hpool.tile([128, BG], f32r, name="h0") for _ in range(NG)]

            pf_sb = consts.tile([128, 4, BC], f32)
            nc.sync.dma_start(out=pf_sb, in_=pf_d.ap().rearrange("g p b -> p g b"))

            # ---- peeled chunk 0 ----
            hs, cs = run_chunk(hs, cs, slice(0, u0), first=True)

            # ---- chunks 1..n_chunks-1 ----
            if n_chunks > 1:
                with tc.For_i(1, n_chunks, 1, staggered_reset=True) as ci:
                    hs, cs = run_chunk(hs, cs, bass.ts(ci, U))
    nc.compile()
    return nc


_NC_CACHE = {}


def _get_nc():
    if "nc" not in _NC_CACHE:
        _NC_CACHE["nc"] = _build_nc()
    return _NC_CACHE["nc"]


def kernel(z, condition, start_point, W_ih, W_hh, b_ih, b_hh, W_out, b_out, seq_len):
    z = np.asarray(z, dtype=np.float32)
    condition = np.asarray(condition, dtype=np.float32)
    start_point = np.asarray(start_point, dtype=np.float32)
    W_ih = np.asarray(W_ih, dtype=np.float32)
    W_hh = np.asarray(W_hh, dtype=np.float32)
    b_ih = np.asarray(b_ih, dtype=np.float32)
    b_hh = np.asarray(b_hh, dtype=np.float32)
    W_out = np.asarray(W_out, dtype=np.float32)
    b_out = np.asarray(b_out, dtype=np.float32)
    assert int(seq_len) == T and z.shape[0] == B_FULL

    B = z.shape[0]
    dt_col = np.full((B, 1), 0.05, dtype=np.float32)
    static_in = np.concatenate([z, condition, dt_col], axis=-1)          # [B, 37]
    static_proj = static_in @ W_ih[:, INPUT_SIZE:].T + b_ih + b_hh       # [B, 512]
    Wx = W_ih[:, :INPUT_SIZE]                                            # [512, 2]
    P0 = static_proj + b_out @ Wx.T                                      # [B, 512]
    Pf = static_proj + start_point @ Wx.T                                # [B, 512]
    W_comb = (W_hh.T + W_out.T @ Wx.T).astype(np.float32)                # [128, 512]

    # unit-major, per-gate: [4, 128, B]; reorder gates [i,f,g,o] -> [i,f,o,g]
    GP = [0, 1, 3, 2]
    P0_t = np.ascontiguousarray(P0.T.reshape(4, HIDDEN, B)[GP], dtype=np.float32)
    Pf_t = np.ascontiguousarray(Pf.T.reshape(4, HIDDEN, B)[GP], dtype=np.float32)
    W_comb = np.ascontiguousarray(
        W_comb.reshape(HIDDEN, 4, HIDDEN)[:, GP, :].reshape(HIDDEN, 4 * HIDDEN)
    )
    woutT = np.ascontiguousarray(W_out.T, dtype=np.float32)              # [128, 2]
    ident = np.eye(HIDDEN, dtype=np.float32)
    bout_rep = np.tile(b_out, U * KT)[None, :].astype(np.float32)

    nc = _get_nc()
    in_maps = []
    for c in range(NCORES):
        bs = slice(c * BC, (c + 1) * BC)
        in_maps.append({
            "p0": np.ascontiguousarray(P0_t[:, :, bs]),
            "pf": np.ascontiguousarray(Pf_t[:, :, bs]),
            "wcomb": W_comb,
            "woutT": woutT,
            "ident": ident,
            "bout": bout_rep,
        })
    global _last_in_maps
    _last_in_maps = in_maps
    res = run_bass_kernel_spmd(nc, in_maps, core_ids=list(range(NCORES)))
    out = np.concatenate([r["y"] for r in res.results], axis=0)
    return out


_last_in_maps = None


# revision 9
# speedup vs baseline: 5.8671x; 5.8671x over previous
"""Trainium2 Bass kernel for nn_CVAE decoder LSTM (v3).

Data-parallel over 8 NeuronCores: batch 8192 -> 1024 per core.

Math (per core, B=1024, T=2048, H=128):
  W_comb = W_hh.T + W_out.T @ Wx.T            [128, 512]
  P0     = static_proj + b_out @ Wx.T         [B, 512]  (constant per step)
  gates_t = h_t @ W_comb + P0                 (t >= 1)
  gates_0 = static_proj + start @ Wx.T = Pf   (h_0 = 0)

v3 layout: hid-on-partition, batch on free. NG=2 groups of BG=512.
Gate order permuted to [f, i, o, g]. Per group per step PSUM tiles
t_fio [128,3,512] (3 banks, bufs=2) and t_g [128,512] (1 bank, bufs=1);
each 1-bank gate slice gets ident@P0 (start) then W@h (accumulate).
x' = h @ WoutT accumulated into a 1-bank PSUM tile x_ps[128,U,8,2]
across a U-step chunk; one bias-add + DMA per chunk.

All elementwise ops on DVE in program order (no inter-engine sems in
the c-chain); sigmoid/tanh outputs in bf16 so u and h run in DVE 2x
mode; c stays fp32. ACT: sig(fio) | tanh(g) | tanh(c).
"""

import os
import numpy as np

import concourse.bass as bass
import concourse.bacc as bacc
import concourse.tile as tile
from concourse import mybir
from concourse.bass_utils import run_bass_kernel_spmd

f32 = mybir.dt.float32
f32r = mybir.dt.float32r
bf16 = mybir.dt.bfloat16
AF = mybir.ActivationFunctionType

HIDDEN = 128
INPUT_SIZE = 2
T = 2048
B_FULL = 8192
NCORES = 8
BC = B_FULL // NCORES      # 1024 batch rows per core
NG = 2                     # batch groups per core
BG = BC // NG              # 512 rows per group
NBT = BG // 128            # 4 batch-tiles of 128 per group
KT = NG * NBT              # 8 batch-tiles of 128 per core
U = int(os.environ.get("K_U", "32"))   # steps per loop chunk (x_ps: U*64B <= 2KB)
SKIPGC = os.environ.get("K_SKIPGC", "1") == "1"
WDT = bf16 if os.environ.get("K_WBF16", "1") == "1" else f32r


def _build_nc(t_total=T):
    nc = bacc.Bacc("TRN2", target_bir_lowering=False)

    wnp = mybir.dt.np(WDT)
    p0_d = nc.dram_tensor("p0", [4, HIDDEN, BC], f32r, kind="ExternalInput")
    pf_d = nc.dram_tensor("pf", [4, HIDDEN, BC], f32, kind="ExternalInput")
    wcomb_d = nc.dram_tensor("wcomb", [HIDDEN, 4 * HIDDEN], WDT, kind="ExternalInput")
    woutT_d = nc.dram_tensor("woutT", [HIDDEN, INPUT_SIZE], WDT, kind="ExternalInput")
    ident_d = nc.dram_tensor("ident", [HIDDEN, HIDDEN], f32r, kind="ExternalInput")
    bout_d = nc.dram_tensor("bout", [1, U * KT * INPUT_SIZE], f32,
                            kind="ExternalInput")
    y_d = nc.dram_tensor("y", [BC, t_total, INPUT_SIZE], f32, kind="ExternalOutput")
    # [p, k, t, j] view of y: batch row b = k*128 + p
    y_v = y_d.ap().rearrange("(k p) t j -> p k t j", p=128)

    u0 = min(U, t_total)
    n_chunks = t_total // U if t_total >= U else 1

    with tile.TileContext(nc) as tc:
        with (
            tc.tile_pool(name="consts", bufs=1) as consts,
            tc.tile_pool(name="hpool", bufs=2 * NG) as hpool,
            tc.tile_pool(name="cpool", bufs=2 * NG) as cpool,
            tc.tile_pool(name="cell", bufs=4 * NG) as cell,
            tc.tile_pool(name="ybuf", bufs=2) as ybuf,
            tc.tile_pool(name="ps_g", bufs=1, space="PSUM") as ps_g,
            tc.tile_pool(name="ps_x", bufs=1, space="PSUM") as ps_x,
        ):
            # ---- constants ----
            wcomb = consts.tile([HIDDEN, 4 * HIDDEN], WDT)
            woutT = consts.tile([HIDDEN, INPUT_SIZE], WDT)
            ident = consts.tile([HIDDEN, HIDDEN], f32r)
            bout = consts.tile([128, U, KT * INPUT_SIZE], f32)
            p0 = consts.tile([128, 4, BC], f32r)   # [hid, gate, batch]
            nc.gpsimd.dma_start(out=wcomb, in_=wcomb_d[:, :])
            nc.gpsimd.dma_start(out=woutT, in_=woutT_d[:, :])
            nc.gpsimd.dma_start(out=ident, in_=ident_d[:, :])
            nc.gpsimd.dma_start(
                out=bout,
                in_=bout_d.ap().rearrange("o (u x) -> o u x", u=U)
                .to_broadcast((128, U, KT * INPUT_SIZE)))
            nc.gpsimd.dma_start(out=p0, in_=p0_d.ap().rearrange("g p b -> p g b"))

            def step(gi, h_prev, c_prev, x_ps, s, pf_sb=None):
                """One LSTM cell step for group gi. Returns (h_new, c_new)."""
                bs = slice(gi * BG, (gi + 1) * BG)
                if pf_sb is None:
                    t_fio = ps_g.tile([128, 3, BG], f32, tag="t_fio", bufs=2)
                    t_g = ps_g.tile([128, BG], f32, tag="t_g", bufs=1)
                    # t_g first: single-buffered, freed earliest by ACT.
                    # Idents (start) issued before W@h: they only need the
                    # bank free, so they run in the h-dependency window.
                    slots = [(t_g, 3), (t_fio[:, 0, :], 0),
                             (t_fio[:, 1, :], 1), (t_fio[:, 2, :], 2)]
                    for out_ap, gg in slots:
                        nc.tensor.matmul(
                            out_ap, ident, p0[:, gg, bs],
                            start=True, stop=False, skip_group_check=SKIPGC)
                    for out_ap, gg in slots:
                        nc.tensor.matmul(
                            out_ap,
                            wcomb[:, gg * 128:(gg + 1) * 128],
                            h_prev[:, :],
                            start=False, stop=True, skip_group_check=SKIPGC)
                else:
                    t_fio = pf_sb[:, 0:3, bs]
                    t_g = pf_sb[:, 3, bs]

                # ACT: sig(f,i,o) merged | tanh(g); all bf16 outputs for
                # DVE 2x modes.
                s_fio = cell.tile([128, 3, BG], bf16, tag="s_fio")
                nc.scalar.activation(s_fio, t_fio, AF.Sigmoid)
                g_t = cell.tile([128, BG], bf16, tag="g_t")
                nc.scalar.activation(g_t, t_g, AF.Tanh)

                # DVE, program order (no inter-engine sems inside the chain):
                # t2 = sig(f)*c ; u = sig(i)*tanh(g) [2x] ; c' = t2 + u
                t2 = cell.tile([128, BG], f32, tag="t2")
                nc.vector.tensor_mul(t2, s_fio[:, 0, :], c_prev)
                u = cell.tile([128, BG], bf16, tag="u")
                nc.vector.tensor_mul(u, s_fio[:, 1, :], g_t)
                c_new = cpool.tile([128, BG], f32)
                nc.vector.tensor_add(c_new, t2, u)

                tau = cell.tile([128, BG], bf16, tag="tau")
                nc.scalar.activation(tau, c_new, AF.Tanh)
                h_new = hpool.tile([128, BG], WDT)
                nc.vector.tensor_mul(h_new, s_fio[:, 2, :], tau)

                # x' = h' @ WoutT into the chunk PSUM accumulator
                for k in range(NBT):
                    nc.tensor.matmul(
                        x_ps[:, s, NBT * gi + k, :],
                        h_new[:, k * 128:(k + 1) * 128],
                        woutT,
                        start=True, stop=True)
                return h_new, c_new

            def run_chunk(hs, cs, t_slice, first=False):
                x_ps_t = ps_x.tile([128, U, KT * INPUT_SIZE], f32)
                x_ps = x_ps_t.rearrange("p u (k j) -> p u k j", j=INPUT_SIZE)
                for s in range(U):
                    new = [step(gi, hs[gi], cs[gi], x_ps, s,
                                pf_sb=(pf_sb if (first and s == 0) else None))
                           for gi in range(NG)]
                    hs = [n[0] for n in new]
                    cs = [n[1] for n in new]
                y_sb = ybuf.tile([128, KT, U, INPUT_SIZE], f32)
                nc.vector.tensor_add(
                    y_sb.rearrange("p k u j -> p u k j"),
                    x_ps,
                    bout.rearrange("p u (k j) -> p u k j", j=INPUT_SIZE))
                nc.sync.dma_start(out=y_v[:, :, t_slice, :], in_=y_sb)
                return hs, cs

            # ---- initial state ----
            cs = []
            for gi in range(NG):
                c0 = cpool.tile([128, BG], f32, name="c0")
                nc.vector.memset(c0, 0.0)
                cs.append(c0)
            hs = [hpool.tile([128, BG], WDT, name="h0") for _ in range(NG)]

            pf_sb = consts.tile([128, 4, BC], f32)
            nc.sync.dma_start(out=pf_sb, in_=pf_d.ap().rearrange("g p b -> p g b"))

            # ---- peeled chunk 0 ----
            hs, cs = run_chunk(hs, cs, slice(0, u0), first=True)

            # ---- chunks 1..n_chunks-1 ----
            if n_chunks > 1:
                with tc.For_i(1, n_chunks, 1, staggered_reset=True) as ci:
                    hs, cs = run_chunk(hs, cs, bass.ts(ci, U))
    nc.compile()
    return nc


_NC_CACHE = {}


def _get_nc():
    if "nc" not in _NC_CACHE:
        _NC_CACHE["nc"] = _build_nc()
    return _NC_CACHE["nc"]


def kernel(z, condition, start_point, W_ih, W_hh, b_ih, b_hh, W_out, b_out, seq_len):
    z = np.asarray(z, dtype=np.float32)
    condition = np.asarray(condition, dtype=np.float32)
    start_point = np.asarray(start_point, dtype=np.float32)
    W_ih = np.asarray(W_ih, dtype=np.float32)
    W_hh = np.asarray(W_hh, dtype=np.float32)
    b_ih = np.asarray(b_ih, dtype=np.float32)
    b_hh = np.asarray(b_hh, dtype=np.float32)
    W_out = np.asarray(W_out, dtype=np.float32)
    b_out = np.asarray(b_out, dtype=np.float32)
    assert int(seq_len) == T and z.shape[0] == B_FULL

    B = z.shape[0]
    dt_col = np.full((B, 1), 0.05, dtype=np.float32)
    static_in = np.concatenate([z, condition, dt_col], axis=-1)          # [B, 37]
    static_proj = static_in @ W_ih[:, INPUT_SIZE:].T + b_ih + b_hh       # [B, 512]
    Wx = W_ih[:, :INPUT_SIZE]                                            # [512, 2]
    P0 = static_proj + b_out @ Wx.T                                      # [B, 512]
    Pf = static_proj + start_point @ Wx.T                                # [B, 512]
    W_comb = (W_hh.T + W_out.T @ Wx.T).astype(np.float32)                # [128, 512]

    # unit-major, per-gate: [4, 128, B]; reorder gates [i,f,g,o] -> [f,i,o,g]
    GP = [1, 0, 3, 2]
    P0_t = np.ascontiguousarray(P0.T.reshape(4, HIDDEN, B)[GP], dtype=np.float32)
    Pf_t = np.ascontiguousarray(Pf.T.reshape(4, HIDDEN, B)[GP], dtype=np.float32)
    W_comb = np.ascontiguousarray(
        W_comb.reshape(HIDDEN, 4, HIDDEN)[:, GP, :].reshape(HIDDEN, 4 * HIDDEN)
    )
    import ml_dtypes
    wnp = np.dtype(ml_dtypes.bfloat16) if WDT == bf16 else np.float32
    woutT = np.ascontiguousarray(W_out.T).astype(wnp)                    # [128, 2]
    ident = np.eye(HIDDEN, dtype=np.float32)
    bout_rep = np.tile(b_out, U * KT)[None, :].astype(np.float32)

    nc = _get_nc()
    in_maps = []
    for c in range(NCORES):
        bs = slice(c * BC, (c + 1) * BC)
        in_maps.append({
            "p0": np.ascontiguousarray(P0_t[:, :, bs]),
            "pf": np.ascontiguousarray(Pf_t[:, :, bs]),
            "wcomb": W_comb.astype(wnp),
            "woutT": woutT,
            "ident": ident,
            "bout": bout_rep,
        })
    global _last_in_maps
    _last_in_maps = in_maps
    res = run_bass_kernel_spmd(nc, in_maps, core_ids=list(range(NCORES)))
    out = np.concatenate([r["y"] for r in res.results], axis=0)
    return out


_last_in_maps = None


# revision 13
# speedup vs baseline: 6.4301x; 1.0960x over previous
"""Trainium2 Bass kernel for nn_CVAE decoder LSTM (v3).

Data-parallel over 8 NeuronCores: batch 8192 -> 1024 per core.

Math (per core, B=1024, T=2048, H=128):
  W_comb = W_hh.T + W_out.T @ Wx.T            [128, 512]
  P0     = static_proj + b_out @ Wx.T         [B, 512]  (constant per step)
  gates_t = h_t @ W_comb + P0                 (t >= 1)
  gates_0 = static_proj + start @ Wx.T = Pf   (h_0 = 0)

v3 layout: hid-on-partition, batch on free. NG=2 groups of BG=512.
Gate order permuted to [f, i, o, g]. Per group per step PSUM tiles
t_fio [128,3,512] (3 banks, bufs=2) and t_g [128,512] (1 bank, bufs=1);
each 1-bank gate slice gets ident@P0 (start) then W@h (accumulate).
x' = h @ WoutT accumulated into a 1-bank PSUM tile x_ps[128,U,8,2]
across a U-step chunk; one bias-add + DMA per chunk.

All elementwise ops on DVE in program order (no inter-engine sems in
the c-chain); sigmoid/tanh outputs in bf16 so u and h run in DVE 2x
mode; c stays fp32. ACT: sig(fio) | tanh(g) | tanh(c).
"""

import os
import numpy as np

import concourse.bass as bass
import concourse.bacc as bacc
import concourse.tile as tile
from concourse import mybir
from concourse.bass_utils import run_bass_kernel_spmd

f32 = mybir.dt.float32
f32r = mybir.dt.float32r
bf16 = mybir.dt.bfloat16
AF = mybir.ActivationFunctionType

HIDDEN = 128
INPUT_SIZE = 2
T = 2048
B_FULL = 8192
NCORES = 8
BC = B_FULL // NCORES      # 1024 batch rows per core
NG = 2                     # batch groups per core
BG = BC // NG              # 512 rows per group
NBT = BG // 128            # 4 batch-tiles of 128 per group
KT = NG * NBT              # 8 batch-tiles of 128 per core
U = int(os.environ.get("K_U", "32"))   # steps per loop chunk (x_ps: U*64B <= 2KB)
SKIPGC = os.environ.get("K_SKIPGC", "1") == "1"
WDT = bf16 if os.environ.get("K_WBF16", "1") == "1" else f32r


def _build_nc(t_total=T):
    nc = bacc.Bacc("TRN2", target_bir_lowering=False)

    wnp = mybir.dt.np(WDT)
    p0_d = nc.dram_tensor("p0", [4, HIDDEN, BC], f32r, kind="ExternalInput")
    pf_d = nc.dram_tensor("pf", [4, HIDDEN, BC], f32, kind="ExternalInput")
    wcomb_d = nc.dram_tensor("wcomb", [HIDDEN, 4 * HIDDEN], WDT, kind="ExternalInput")
    woutT_d = nc.dram_tensor("woutT", [HIDDEN, INPUT_SIZE], WDT, kind="ExternalInput")
    ident_d = nc.dram_tensor("ident", [HIDDEN, HIDDEN], f32r, kind="ExternalInput")
    bout_d = nc.dram_tensor("bout", [1, U * KT * INPUT_SIZE], f32,
                            kind="ExternalInput")
    y_d = nc.dram_tensor("y", [BC, t_total, INPUT_SIZE], f32, kind="ExternalOutput")
    # [p, k, t, j] view of y: batch row b = k*128 + p
    y_v = y_d.ap().rearrange("(k p) t j -> p k t j", p=128)

    u0 = min(U, t_total)
    n_chunks = t_total // U if t_total >= U else 1

    with tile.TileContext(nc) as tc:
        with (
            tc.tile_pool(name="consts", bufs=1) as consts,
            tc.tile_pool(name="hpool", bufs=3 * NG) as hpool,
            tc.tile_pool(name="cpool", bufs=3 * NG) as cpool,
            tc.tile_pool(name="cell", bufs=6 * NG) as cell,
            tc.tile_pool(name="ybuf", bufs=2) as ybuf,
            tc.tile_pool(name="ps_g", bufs=1, space="PSUM") as ps_g,
            tc.tile_pool(name="ps_x", bufs=1, space="PSUM") as ps_x,
        ):
            # ---- constants ----
            wcomb = consts.tile([HIDDEN, 4 * HIDDEN], WDT)
            woutT = consts.tile([HIDDEN, INPUT_SIZE], WDT)
            ident = consts.tile([HIDDEN, HIDDEN], f32r)
            bout = consts.tile([128, U, KT * INPUT_SIZE], f32)
            p0 = consts.tile([128, 4, BC], f32r)   # [hid, gate, batch]
            nc.gpsimd.dma_start(out=wcomb, in_=wcomb_d[:, :])
            nc.gpsimd.dma_start(out=woutT, in_=woutT_d[:, :])
            nc.gpsimd.dma_start(out=ident, in_=ident_d[:, :])
            nc.gpsimd.dma_start(
                out=bout,
                in_=bout_d.ap().rearrange("o (u x) -> o u x", u=U)
                .to_broadcast((128, U, KT * INPUT_SIZE)))
            nc.gpsimd.dma_start(out=p0, in_=p0_d.ap().rearrange("g p b -> p g b"))

            def step(gi, h_prev, c_prev, x_ps, s, pf_sb=None):
                """One LSTM cell step for group gi. Returns (h_new, c_new)."""
                bs = slice(gi * BG, (gi + 1) * BG)
                if pf_sb is None:
                    t_fio = ps_g.tile([128, 3, BG], f32, tag="t_fio", bufs=2)
                    t_g = ps_g.tile([128, BG], f32, tag="t_g", bufs=1)
                    # t_g first: single-buffered, freed earliest by ACT.
                    # Idents (start) issued before W@h: they only need the
                    # bank free, so they run in the h-dependency window.
                    slots = [(t_g, 3), (t_fio[:, 0, :], 0),
                             (t_fio[:, 1, :], 1), (t_fio[:, 2, :], 2)]
                    for out_ap, gg in slots:
                        nc.tensor.matmul(
                            out_ap, ident, p0[:, gg, bs],
                            start=True, stop=False, skip_group_check=SKIPGC)
                    for out_ap, gg in slots:
                        nc.tensor.matmul(
                            out_ap,
                            wcomb[:, gg * 128:(gg + 1) * 128],
                            h_prev[:, :],
                            start=False, stop=True, skip_group_check=SKIPGC)
                else:
                    t_fio = pf_sb[:, 0:3, bs]
                    t_g = pf_sb[:, 3, bs]

                # ACT: sig(f,i,o) merged | tanh(g); all bf16 outputs for
                # DVE 2x modes.
                s_fio = cell.tile([128, 3, BG], bf16, tag="s_fio")
                nc.scalar.activation(s_fio, t_fio, AF.Sigmoid)
                g_t = cell.tile([128, BG], bf16, tag="g_t")
                nc.scalar.activation(g_t, t_g, AF.Tanh)

                # DVE, program order (no inter-engine sems inside the chain):
                # t2 = sig(f)*c ; u = sig(i)*tanh(g) [2x] ; c' = t2 + u
                t2 = cell.tile([128, BG], f32, tag="t2")
                nc.vector.tensor_mul(t2, s_fio[:, 0, :], c_prev)
                u = cell.tile([128, BG], bf16, tag="u")
                nc.vector.tensor_mul(u, s_fio[:, 1, :], g_t)
                c_new = cpool.tile([128, BG], f32)
                nc.vector.tensor_add(c_new, t2, u)

                tau = cell.tile([128, BG], bf16, tag="tau")
                nc.scalar.activation(tau, c_new, AF.Tanh)
                h_new = hpool.tile([128, BG], WDT)
                nc.vector.tensor_mul(h_new, s_fio[:, 2, :], tau)
                return h_new, c_new

            def emit_x(gi, h_new, x_ps, s):
                # x' = h' @ WoutT into the chunk PSUM accumulator. Issued
                # after BOTH groups' gate matmuls: x matmuls wait on late h,
                # and the PE FIFO is in-order - putting them inside step()
                # would head-of-line-block the other group's ident/gate
                # matmuls and serialize the two chains.
                for k in range(NBT):
                    nc.tensor.matmul(
                        x_ps[:, s, NBT * gi + k, :],
                        h_new[:, k * 128:(k + 1) * 128],
                        woutT,
                        start=True, stop=True)

            def run_chunk(hs, cs, t_slice, first=False):
                x_ps_t = ps_x.tile([128, U, KT * INPUT_SIZE], f32)
                x_ps = x_ps_t.rearrange("p u (k j) -> p u k j", j=INPUT_SIZE)
                for s in range(U):
                    new = [step(gi, hs[gi], cs[gi], x_ps, s,
                                pf_sb=(pf_sb if (first and s == 0) else None))
                           for gi in range(NG)]
                    hs = [n[0] for n in new]
                    cs = [n[1] for n in new]
                    for gi in range(NG):
                        emit_x(gi, hs[gi], x_ps, s)
                y_sb = ybuf.tile([128, KT, U, INPUT_SIZE], f32)
                nc.vector.tensor_add(
                    y_sb.rearrange("p k u j -> p u k j"),
                    x_ps,
                    bout.rearrange("p u (k j) -> p u k j", j=INPUT_SIZE))
                nc.sync.dma_start(out=y_v[:, :, t_slice, :], in_=y_sb)
                return hs, cs

            # ---- initial state ----
            cs = []
            for gi in range(NG):
                c0 = cpool.tile([128, BG], f32, name="c0")
                nc.vector.memset(c0, 0.0)
                cs.append(c0)
            hs = [hpool.tile([128, BG], WDT, name="h0") for _ in range(NG)]

            pf_sb = consts.tile([128, 4, BC], f32)
            nc.sync.dma_start(out=pf_sb, in_=pf_d.ap().rearrange("g p b -> p g b"))

            # ---- peeled chunk 0 ----
            hs, cs = run_chunk(hs, cs, slice(0, u0), first=True)

            # ---- chunks 1..n_chunks-1 ----
            if n_chunks > 1:
                with tc.For_i(1, n_chunks, 1, staggered_reset=True) as ci:
                    hs, cs = run_chunk(hs, cs, bass.ts(ci, U))
    nc.compile()
    return nc


_NC_CACHE = {}


def _get_nc():
    if "nc" not in _NC_CACHE:
        _NC_CACHE["nc"] = _build_nc()
    return _NC_CACHE["nc"]


def kernel(z, condition, start_point, W_ih, W_hh, b_ih, b_hh, W_out, b_out, seq_len):
    z = np.asarray(z, dtype=np.float32)
    condition = np.asarray(condition, dtype=np.float32)
    start_point = np.asarray(start_point, dtype=np.float32)
    W_ih = np.asarray(W_ih, dtype=np.float32)
    W_hh = np.asarray(W_hh, dtype=np.float32)
    b_ih = np.asarray(b_ih, dtype=np.float32)
    b_hh = np.asarray(b_hh, dtype=np.float32)
    W_out = np.asarray(W_out, dtype=np.float32)
    b_out = np.asarray(b_out, dtype=np.float32)
    assert int(seq_len) == T and z.shape[0] == B_FULL

    B = z.shape[0]
    dt_col = np.full((B, 1), 0.05, dtype=np.float32)
    static_in = np.concatenate([z, condition, dt_col], axis=-1)          # [B, 37]
    static_proj = static_in @ W_ih[:, INPUT_SIZE:].T + b_ih + b_hh       # [B, 512]
    Wx = W_ih[:, :INPUT_SIZE]                                            # [512, 2]
    P0 = static_proj + b_out @ Wx.T                                      # [B, 512]
    Pf = static_proj + start_point @ Wx.T                                # [B, 512]
    W_comb = (W_hh.T + W_out.T @ Wx.T).astype(np.float32)                # [128, 512]

    # unit-major, per-gate: [4, 128, B]; reorder gates [i,f,g,o] -> [f,i,o,g]
    GP = [1, 0, 3, 2]
    P0_t = np.ascontiguousarray(P0.T.reshape(4, HIDDEN, B)[GP], dtype=np.float32)
    Pf_t = np.ascontiguousarray(Pf.T.reshape(4, HIDDEN, B)[GP], dtype=np.float32)
    W_comb = np.ascontiguousarray(
        W_comb.reshape(HIDDEN, 4, HIDDEN)[:, GP, :].reshape(HIDDEN, 4 * HIDDEN)
    )
    import ml_dtypes
    wnp = np.dtype(ml_dtypes.bfloat16) if WDT == bf16 else np.float32
    woutT = np.ascontiguousarray(W_out.T).astype(wnp)                    # [128, 2]
    ident = np.eye(HIDDEN, dtype=np.float32)
    bout_rep = np.tile(b_out, U * KT)[None, :].astype(np.float32)

    nc = _get_nc()
    in_maps = []
    for c in range(NCORES):
        bs = slice(c * BC, (c + 1) * BC)
        in_maps.append({
            "p0": np.ascontiguousarray(P0_t[:, :, bs]),
            "pf": np.ascontiguousarray(Pf_t[:, :, bs]),
            "wcomb": W_comb.astype(wnp),
            "woutT": woutT,
            "ident": ident,
            "bout": bout_rep,
        })
    global _last_in_maps
    _last_in_maps = in_maps
    res = run_bass_kernel_spmd(nc, in_maps, core_ids=list(range(NCORES)))
    out = np.concatenate([r["y"] for r in res.results], axis=0)
    return out


_last_in_maps = None


# revision 15
# speedup vs baseline: 6.6843x; 1.0395x over previous
"""Trainium2 Bass kernel for nn_CVAE decoder LSTM (v3).

Data-parallel over 8 NeuronCores: batch 8192 -> 1024 per core.

Math (per core, B=1024, T=2048, H=128):
  W_comb = W_hh.T + W_out.T @ Wx.T            [128, 512]
  P0     = static_proj + b_out @ Wx.T         [B, 512]  (constant per step)
  gates_t = h_t @ W_comb + P0                 (t >= 1)
  gates_0 = static_proj + start @ Wx.T = Pf   (h_0 = 0)

v3 layout: hid-on-partition, batch on free. NG=2 groups of BG=512.
Gate order permuted to [f, i, o, g]. Per group per step PSUM tiles
t_fio [128,3,512] (3 banks, bufs=2) and t_g [128,512] (1 bank, bufs=1);
each 1-bank gate slice gets ident@P0 (start) then W@h (accumulate).
x' = h @ WoutT accumulated into a 1-bank PSUM tile x_ps[128,U,8,2]
across a U-step chunk; one bias-add + DMA per chunk.

All elementwise ops on DVE in program order (no inter-engine sems in
the c-chain); sigmoid/tanh outputs in bf16 so u and h run in DVE 2x
mode; c stays fp32. ACT: sig(fio) | tanh(g) | tanh(c).
"""

import os
import numpy as np

import concourse.bass as bass
import concourse.bacc as bacc
import concourse.tile as tile
from concourse import mybir
from concourse.bass_utils import run_bass_kernel_spmd

f32 = mybir.dt.float32
f32r = mybir.dt.float32r
bf16 = mybir.dt.bfloat16
AF = mybir.ActivationFunctionType

HIDDEN = 128
INPUT_SIZE = 2
T = 2048
B_FULL = 8192
NCORES = 8
BC = B_FULL // NCORES      # 1024 batch rows per core
NG = 2                     # batch groups per core
BG = BC // NG              # 512 rows per group
NBT = BG // 128            # 4 batch-tiles of 128 per group
KT = NG * NBT              # 8 batch-tiles of 128 per core
U = int(os.environ.get("K_U", "32"))   # steps per loop chunk (x_ps: U*64B <= 2KB)
SKIPGC = os.environ.get("K_SKIPGC", "1") == "1"
WDT = bf16 if os.environ.get("K_WBF16", "1") == "1" else f32r


def _build_nc(t_total=T):
    nc = bacc.Bacc("TRN2", target_bir_lowering=False)

    wnp = mybir.dt.np(WDT)
    p0_d = nc.dram_tensor("p0", [4, HIDDEN, BC], f32r, kind="ExternalInput")
    pf_d = nc.dram_tensor("pf", [4, HIDDEN, BC], f32, kind="ExternalInput")
    wcomb_d = nc.dram_tensor("wcomb", [HIDDEN, 4 * HIDDEN], WDT, kind="ExternalInput")
    woutT_d = nc.dram_tensor("woutT", [HIDDEN, INPUT_SIZE], WDT, kind="ExternalInput")
    ident_d = nc.dram_tensor("ident", [HIDDEN, HIDDEN], f32r, kind="ExternalInput")
    bout_d = nc.dram_tensor("bout", [1, U * KT * INPUT_SIZE], f32,
                            kind="ExternalInput")
    y_d = nc.dram_tensor("y", [BC, t_total, INPUT_SIZE], f32, kind="ExternalOutput")
    # [p, k, t, j] view of y: batch row b = k*128 + p
    y_v = y_d.ap().rearrange("(k p) t j -> p k t j", p=128)

    u0 = min(U, t_total)
    n_chunks = t_total // U if t_total >= U else 1

    with tile.TileContext(nc) as tc:
        with (
            tc.tile_pool(name="consts", bufs=1) as consts,
            tc.tile_pool(name="hpool", bufs=3 * NG) as hpool,
            tc.tile_pool(name="cpool", bufs=3 * NG) as cpool,
            tc.tile_pool(name="cell", bufs=6 * NG) as cell,
            tc.tile_pool(name="ybuf", bufs=2) as ybuf,
            tc.tile_pool(name="ps_g", bufs=1, space="PSUM") as ps_g,
            tc.tile_pool(name="ps_x", bufs=1, space="PSUM") as ps_x,
        ):
            # ---- constants ----
            wcomb = consts.tile([HIDDEN, 4 * HIDDEN], WDT)
            woutT = consts.tile([HIDDEN, INPUT_SIZE], WDT)
            ident = consts.tile([HIDDEN, HIDDEN], f32r)
            bout = consts.tile([128, U, KT * INPUT_SIZE], f32)
            p0 = consts.tile([128, 4, BC], f32r)   # [hid, gate, batch]
            nc.gpsimd.dma_start(out=wcomb, in_=wcomb_d[:, :])
            nc.gpsimd.dma_start(out=woutT, in_=woutT_d[:, :])
            nc.gpsimd.dma_start(out=ident, in_=ident_d[:, :])
            nc.gpsimd.dma_start(
                out=bout,
                in_=bout_d.ap().rearrange("o (u x) -> o u x", u=U)
                .to_broadcast((128, U, KT * INPUT_SIZE)))
            nc.gpsimd.dma_start(out=p0, in_=p0_d.ap().rearrange("g p b -> p g b"))

            def step(gi, h_prev, c_prev, x_ps, s, pf_sb=None):
                """One LSTM cell step for group gi. Returns (h_new, c_new)."""
                bs = slice(gi * BG, (gi + 1) * BG)
                if pf_sb is None:
                    t_fio = ps_g.tile([128, 3, BG], f32, tag="t_fio", bufs=2)
                    t_g = ps_g.tile([128, BG], f32, tag="t_g", bufs=1)
                    # t_g first: single-buffered, freed earliest by ACT.
                    # Idents (start) issued before W@h: they only need the
                    # bank free, so they run in the h-dependency window.
                    slots = [(t_g, 3), (t_fio[:, 0, :], 0),
                             (t_fio[:, 1, :], 1), (t_fio[:, 2, :], 2)]
                    for out_ap, gg in slots:
                        nc.tensor.matmul(
                            out_ap, ident, p0[:, gg, bs],
                            start=True, stop=False, skip_group_check=SKIPGC)
                    for out_ap, gg in slots:
                        nc.tensor.matmul(
                            out_ap,
                            wcomb[:, gg * 128:(gg + 1) * 128],
                            h_prev[:, :],
                            start=False, stop=True, skip_group_check=SKIPGC)
                else:
                    t_fio = pf_sb[:, 0:3, bs]
                    t_g = pf_sb[:, 3, bs]

                # ACT: sig(f,i,o) merged | tanh(g); all bf16 outputs for
                # DVE 2x modes.
                s_fio = cell.tile([128, 3, BG], bf16, tag="s_fio")
                nc.scalar.activation(s_fio, t_fio, AF.Sigmoid)
                g_t = cell.tile([128, BG], bf16, tag="g_t")
                nc.scalar.activation(g_t, t_g, AF.Tanh)

                # DVE, program order (no inter-engine sems inside the chain):
                # t2 = sig(f)*c ; u = sig(i)*tanh(g) [2x] ; c' = t2 + u
                t2 = cell.tile([128, BG], f32, tag="t2")
                nc.vector.tensor_mul(t2, s_fio[:, 0, :], c_prev)
                u = cell.tile([128, BG], bf16, tag="u")
                nc.vector.tensor_mul(u, s_fio[:, 1, :], g_t)
                c_new = cpool.tile([128, BG], f32)
                nc.vector.tensor_add(c_new, t2, u)

                # chain tail gets scheduler priority: when tanh(c) and the
                # other group's sigmoid are both ready, ACT must take the
                # loop-carried tanh(c) first.
                tau = cell.tile([128, BG], bf16, tag="tau")
                h_new = hpool.tile([128, BG], WDT)
                with tc.high_priority(200):
                    nc.scalar.activation(tau, c_new, AF.Tanh)
                    nc.vector.tensor_mul(h_new, s_fio[:, 2, :], tau)
                return h_new, c_new

            def emit_x(gi, h_new, x_ps, s):
                # x' = h' @ WoutT into the chunk PSUM accumulator. Issued
                # after BOTH groups' gate matmuls: x matmuls wait on late h,
                # and the PE FIFO is in-order - putting them inside step()
                # would head-of-line-block the other group's ident/gate
                # matmuls and serialize the two chains.
                for k in range(NBT):
                    nc.tensor.matmul(
                        x_ps[:, s, NBT * gi + k, :],
                        h_new[:, k * 128:(k + 1) * 128],
                        woutT,
                        start=True, stop=True)

            def run_chunk(hs, cs, t_slice, first=False):
                x_ps_t = ps_x.tile([128, U, KT * INPUT_SIZE], f32)
                x_ps = x_ps_t.rearrange("p u (k j) -> p u k j", j=INPUT_SIZE)
                for s in range(U):
                    new = [step(gi, hs[gi], cs[gi], x_ps, s,
                                pf_sb=(pf_sb if (first and s == 0) else None))
                           for gi in range(NG)]
                    hs = [n[0] for n in new]
                    cs = [n[1] for n in new]
                    for gi in range(NG):
                        emit_x(gi, hs[gi], x_ps, s)
                y_sb = ybuf.tile([128, KT, U, INPUT_SIZE], f32)
                nc.vector.tensor_add(
                    y_sb.rearrange("p k u j -> p u k j"),
                    x_ps,
                    bout.rearrange("p u (k j) -> p u k j", j=INPUT_SIZE))
                nc.sync.dma_start(out=y_v[:, :, t_slice, :], in_=y_sb)
                return hs, cs

            # ---- initial state ----
            cs = []
            for gi in range(NG):
                c0 = cpool.tile([128, BG], f32, name="c0")
                nc.vector.memset(c0, 0.0)
                cs.append(c0)
            hs = [hpool.tile([128, BG], WDT, name="h0") for _ in range(NG)]

            pf_sb = consts.tile([128, 4, BC], f32)
            nc.sync.dma_start(out=pf_sb, in_=pf_d.ap().rearrange("g p b -> p g b"))

            # ---- peeled chunk 0 ----
            hs, cs = run_chunk(hs, cs, slice(0, u0), first=True)

            # ---- chunks 1..n_chunks-1 ----
            if n_chunks > 1:
                with tc.For_i(1, n_chunks, 1, staggered_reset=True) as ci:
                    hs, cs = run_chunk(hs, cs, bass.ts(ci, U))
    nc.compile()
    return nc


_NC_CACHE = {}


def _get_nc():
    if "nc" not in _NC_CACHE:
        _NC_CACHE["nc"] = _build_nc()
    return _NC_CACHE["nc"]


def kernel(z, condition, start_point, W_ih, W_hh, b_ih, b_hh, W_out, b_out, seq_len):
    z = np.asarray(z, dtype=np.float32)
    condition = np.asarray(condition, dtype=np.float32)
    start_point = np.asarray(start_point, dtype=np.float32)
    W_ih = np.asarray(W_ih, dtype=np.float32)
    W_hh = np.asarray(W_hh, dtype=np.float32)
    b_ih = np.asarray(b_ih, dtype=np.float32)
    b_hh = np.asarray(b_hh, dtype=np.float32)
    W_out = np.asarray(W_out, dtype=np.float32)
    b_out = np.asarray(b_out, dtype=np.float32)
    assert int(seq_len) == T and z.shape[0] == B_FULL

    B = z.shape[0]
    dt_col = np.full((B, 1), 0.05, dtype=np.float32)
    static_in = np.concatenate([z, condition, dt_col], axis=-1)          # [B, 37]
    static_proj = static_in @ W_ih[:, INPUT_SIZE:].T + b_ih + b_hh       # [B, 512]
    Wx = W_ih[:, :INPUT_SIZE]                                            # [512, 2]
    P0 = static_proj + b_out @ Wx.T                                      # [B, 512]
    Pf = static_proj + start_point @ Wx.T                                # [B, 512]
    W_comb = (W_hh.T + W_out.T @ Wx.T).astype(np.float32)                # [128, 512]

    # unit-major, per-gate: [4, 128, B]; reorder gates [i,f,g,o] -> [f,i,o,g]
    GP = [1, 0, 3, 2]
    P0_t = np.ascontiguousarray(P0.T.reshape(4, HIDDEN, B)[GP], dtype=np.float32)
    Pf_t = np.ascontiguousarray(Pf.T.reshape(4, HIDDEN, B)[GP], dtype=np.float32)
    W_comb = np.ascontiguousarray(
        W_comb.reshape(HIDDEN, 4, HIDDEN)[:, GP, :].reshape(HIDDEN, 4 * HIDDEN)
    )
    import ml_dtypes
    wnp = np.dtype(ml_dtypes.bfloat16) if WDT == bf16 else np.float32
    woutT = np.ascontiguousarray(W_out.T).astype(wnp)                    # [128, 2]
    ident = np.eye(HIDDEN, dtype=np.float32)
    bout_rep = np.tile(b_out, U * KT)[None, :].astype(np.float32)

    nc = _get_nc()
    in_maps = []
    for c in range(NCORES):
        bs = slice(c * BC, (c + 1) * BC)
        in_maps.append({
            "p0": np.ascontiguousarray(P0_t[:, :, bs]),
            "pf": np.ascontiguousarray(Pf_t[:, :, bs]),
            "wcomb": W_comb.astype(wnp),
            "woutT": woutT,
            "ident": ident,
            "bout": bout_rep,
        })
    global _last_in_maps
    _last_in_maps = in_maps
    res = run_bass_kernel_spmd(nc, in_maps, core_ids=list(range(NCORES)))
    out = np.concatenate([r["y"] for r in res.results], axis=0)
    return out


_last_in_maps = None
